# revision 24
# baseline (speedup 1.0000x reference)
"""Trainium2 Bass kernel for nn_ContrastLoss (smooth-histogram contrast loss).

Algorithm
---------
reference computes, per image:  hist[b] = sum_p w(x_p,b) / (S_p + 1e-8),
w = exp(-0.5*((x - c_b)/sigma)^2), c_b = b/255, sigma = 0.01, S_p = sum_b w,
followed by MSEs between the three histograms.

hist is a fixed linear map of the count histogram of u = round(x * 255)
in [0, 255] (256 levels = the bin centers themselves; quantization error on
the loss is ~5e-4 rel, far inside tolerance):
    hist[b] = sum_u cnt[u] * Phi[u, b]
The device only needs cnt[256] per image — a pure counting problem.

Device kernel (SPMD over 8 cores, data-parallel over pixels):
  - per core/image, 32768 pixels in SBUF [128, 256]; u = round(255 x) via the
    2^23 magic-add on ACT; split u = 16*hi + lo (hi via a second magic-add on
    ACT, lo via one DVE scalar_tensor_tensor, both exact small ints in bf16).
  - counting via PE outer products, NG=8 pixel columns block-diagonal per
    matmul m: ps += onehot_lo(group m)^T @ onehot_hi(cols of m).
    Weights APs must collapse to ONE packed free dim, so onehot_lo lives as
    Olo[p, m, l, g] (l-major inside each 8-column group): [8,16]x[1,8]
    collapses to a 128-long stride-1 run.  The moving operand tolerates a
    strided AP, so onehot_hi lives column-last as Ohi[p, w, c].  The PSUM
    table comes out index-permuted (ps[8l+g, 16g+h]) — host unscrambles.
  - BOTH one-hot layouts give batched DVE is_equal instructions whose
    operands are all 2-byte, SBUF, innermost-stride-1 -> DVE 2x_1p perf mode
    (0.52 ns/elem).  Pool builds the last 40 hi columns via per-column
    tensor_scalar (f32 comparand) to offload DVE; ACT only does prep + the
    PSUM->SBUF copy.
  - DMAs: one tiny f32 iota seed (issued on the DVE queue, which then
    derives the bf16 iota tiles on-device during its idle head), x image 0
    alone (critical path) then images 1+2 in one DMACopy — each DMACopy
    costs ~625ns on the shared HWDGE device, so fewer + smaller is faster.
  - host sums the 8 diagonal blocks of the permuted table (and the 8 cores —
    the all-reduce), applies the exact f64 cell-averaged Phi map, then MSE.
"""

import os
import sys

import numpy as np

for _p in ("/opt/trn_rl_repo", "/root/.axon_site/_ro/trn_rl_repo"):
    if os.path.isdir(_p) and _p not in sys.path:
        sys.path.insert(0, _p)

import concourse.bass as bass  # noqa: E402
import concourse.tile as tile  # noqa: E402
from concourse import bacc, mybir  # noqa: E402
from concourse.bass_utils import run_bass_kernel_spmd, axon_active  # noqa: E402

N_CORES = 8
N_IMG = 3
IMG_PIX = 4 * 1 * 256 * 256          # 262144 pixels per image
SHARD = IMG_PIX // N_CORES           # 32768 pixels per core per image
P, T = 128, 256                      # on-chip pixel layout (SHARD = P*T)
W = 16                               # one-hot width (hi and lo)
NG = 8                               # pixel columns per matmul (block-diag)
NGRP = T // NG                       # 32 column groups per image
GRID = W * W                         # 256 fine levels, u = W*hi + lo
SCALE = 255.0                        # u = round(x * 255): exactly the bins
MAGIC = 8388608.0                    # 2**23: float32 round-to-nearest trick
TC = 108                             # hi columns per DVE build instruction
G_COLS = (56, 40, 24)                # hi columns built on Pool, per image
MCHUNK = 16                          # lo groups per DVE build instruction
SIGMA = 0.01
BINS = 256

_CACHE = {}


def _build_program():
    nc = bacc.Bacc(
        "TRN2",
        target_bir_lowering=False,
        debug=not axon_active(),
        num_devices=N_CORES,
    )
    f32 = mybir.dt.float32
    bf16 = mybir.dt.bfloat16
    A = mybir.AluOpType
    CP = mybir.ActivationFunctionType.Copy

    x_d = nc.dram_tensor("x", [N_IMG, P, T], f32, kind="ExternalInput")
    cnt_d = nc.dram_tensor("cnt", [N_IMG, NG * W, NG * W], f32, kind="ExternalOutput")

    with tile.TileContext(nc) as tc:
        with (
            tc.tile_pool(name="pool", bufs=3) as pool,
            tc.tile_pool(name="cpool", bufs=1) as cpool,
            tc.tile_pool(name="psum", bufs=2, space=bass.MemorySpace.PSUM) as pp,
        ):
            # no iota DMA at all: Pool memsets the 16-wide bf16 seed during
            # its idle head, DVE derives the other iota tiles from it.
            iotaWb = cpool.tile([P, W], bf16, tag="iotaWb")
            for w in range(W):
                nc.gpsimd.memset(iotaWb[:, w : w + 1], float(w))
            iotaRs = cpool.tile([P, W], f32, tag="iotaRs")
            nc.vector.tensor_scalar(iotaRs[:], iotaWb[:], 1.0, None, A.mult)
            iotaL2 = cpool.tile([P, W, NG], bf16, tag="iotaL2")
            nc.vector.tensor_scalar(
                iotaL2[:],
                iotaWb[:, :, None].broadcast_to([P, W, NG]),
                1.0, None, A.mult,
            )
            # iotaH[:, :, 0:8] == iotaL2 content; double out to TC cols
            iotaH = cpool.tile([P, W, TC], bf16, tag="iotaH")
            nc.vector.tensor_scalar(iotaH[:, :, 0:NG], iotaL2[:], 1.0, None, A.mult)
            w_done = NG
            while w_done < TC:
                n = min(w_done, TC - w_done)
                nc.vector.tensor_scalar(
                    iotaH[:, :, w_done : w_done + n],
                    iotaH[:, :, 0:n],
                    1.0, None, A.mult,
                )
                w_done += n

            xs, hfs, hbs, lbs = {}, {}, {}, {}
            for i in range(N_IMG):
                xs[i] = pool.tile([P, T], f32, tag="x", name=f"x{i}")
                if i == 0:
                    nc.sync.dma_start(xs[i][:, 0 : T // 2], x_d[i, :, 0 : T // 2])
                    nc.sync.dma_start(xs[i][:, T // 2 :], x_d[i, :, T // 2 :])
                else:
                    nc.sync.dma_start(xs[i][:], x_d[i])

            # hf/hb/lb are dicts keyed (i, c0): a separate physical tile
            # per prep slice, so consumers of one slice never wait on the
            # tile's other writers (the framework coarsens multi-writer
            # tiles to last-write granularity).
            def stage_a_part(i, c0, c1):
                # u = round(x*255), hi = round((u-7.5)/16) (magic-adds, ACT);
                # lo = u - 16*hi (DVE stt).  hi kept in f32 (Pool comparand)
                # and bf16 (DVE comparand); lo in bf16.
                n, s = c1 - c0, slice(c0, c1)
                nm = f"{i}_{c0}"
                t0 = pool.tile([P, n], f32, tag="t0", name=f"t0_{nm}")
                u = pool.tile([P, n], f32, tag="u", name=f"u_{nm}")
                t1 = pool.tile([P, n], f32, tag="t1", name=f"t1_{nm}")
                t2 = pool.tile([P, n], f32, tag="t2", name=f"t2_{nm}")
                hfs[i, c0] = pool.tile([P, n], f32, tag="hf", name=f"hf_{nm}")
                hbs[i, c0] = pool.tile([P, n], bf16, tag="hb", name=f"hb_{nm}")
                lbs[i, c0] = pool.tile([P, n], bf16, tag="lb", name=f"lb_{nm}")
                nc.scalar.activation(t0[:], xs[i][:, s], CP, bias=MAGIC, scale=SCALE)
                nc.scalar.activation(u[:], t0[:], CP, bias=-MAGIC)
                nc.scalar.activation(
                    t1[:], u[:], CP, bias=8.0 - (W / 2.0 - 0.5) / W, scale=1.0 / W
                )
                nc.scalar.activation(t2[:], t1[:], CP, bias=MAGIC)
                nc.scalar.activation(hfs[i, c0][:], t2[:], CP, bias=-(MAGIC + 8.0))
                nc.scalar.activation(hbs[i, c0][:], hfs[i, c0][:], CP, bias=0.0)
                nc.vector.scalar_tensor_tensor(
                    lbs[i, c0][:], hfs[i, c0][:], -float(W), u[:], A.mult, A.add
                )

            def prep_view(d, i, c0, c1):
                # view of [c0, c1) out of the prep slice tiles covering it
                for (j, s0), tile_ in sorted(d.items()):
                    if j == i and s0 <= c0 and c1 - s0 <= tile_.shape[-1]:
                        return tile_[:, c0 - s0 : c1 - s0]
                raise KeyError((i, c0, c1))

            def olo_chunk(i, m0, m1):
                lbg = prep_view(lbs, i, m0 * NG, m1 * NG).rearrange(
                    "p (m g) -> p m g", g=NG
                )
                nc.vector.tensor_tensor(
                    olos[i][:, m0:m1, :, :],
                    iotaL2[:, None, :, :].broadcast_to([P, m1 - m0, W, NG]),
                    lbg[:, :, None, :].broadcast_to([P, m1 - m0, W, NG]),
                    A.is_equal,
                )

            def hi_chunk(i, c0, c1):
                hb = prep_view(hbs, i, c0, c1)
                nc.vector.tensor_tensor(
                    ohis[i][:, :, c0:c1],
                    iotaH[:, :, 0 : c1 - c0],
                    hb[:, None, :].broadcast_to([P, W, c1 - c0]),
                    A.is_equal,
                )

            def pool_cols(i, c0, c1):
                for c in range(c0, c1):
                    hf = prep_view(hfs, i, c, c + 1)
                    nc.gpsimd.tensor_scalar(
                        ohis[i][:, :, c], iotaRs[:], hf,
                        None, A.is_equal,
                    )

            def finish(i):
                ps = pp.tile([NG * W, NG * W], f32, tag="ps", name=f"ps{i}")
                for m in range(NGRP):
                    lhsT = olos[i][:, m, :, :]
                    rhs = ohis[i][:, :, m * NG : (m + 1) * NG].rearrange(
                        "p w c -> p c w"
                    )
                    nc.tensor.matmul(
                        ps[:], lhsT, rhs, start=(m == 0), stop=(m == NGRP - 1)
                    )
                res = pool.tile([NG * W, NG * W], f32, tag="res", name=f"res{i}")
                nc.scalar.activation(res[:], ps[:], CP, bias=0.0)
                nc.sync.dma_start(cnt_d[i], res[:])

            olos, ohis = {}, {}
            for i in range(N_IMG):
                olos[i] = pool.tile([P, NGRP, W, NG], bf16, tag="Olo", name=f"Olo{i}")
                ohis[i] = pool.tile([P, W, T], bf16, tag="Ohi", name=f"Ohi{i}")

            def stage_prep(i, splits):
                for c0, c1 in splits:
                    stage_a_part(i, c0, c1)

            def stage_bc(i):
                # Pool block at the front for early images (starts as soon
                # as the first hf slice lands), at the back for the last
                # image (final matmuls DVE-gated -> short PE tail).
                gcols = G_COLS[i]
                if i < N_IMG - 1:
                    pool_cols(i, 0, gcols)
                    d0, d1 = gcols, T
                else:
                    pool_cols(i, T - gcols, T)
                    d0, d1 = 0, T - gcols
                olo_chunk(i, 0, NGRP // 2)
                olo_chunk(i, NGRP // 2, NGRP)
                for c0 in range(d0, d1, TC):
                    hi_chunk(i, c0, min(c0 + TC, d1))
                finish(i)

            # image 0 interleaved at half-tile granularity: each prep half
            # immediately unblocks the builds that only need that half.
            g0 = G_COLS[0]
            stage_a_part(0, 0, T // 2)
            pool_cols(0, 0, g0)
            olo_chunk(0, 0, NGRP // 2)
            hi_chunk(0, g0, T // 2)
            stage_a_part(0, T // 2, T)
            olo_chunk(0, NGRP // 2, NGRP)
            for c0 in range(T // 2, T, TC):
                hi_chunk(0, c0, min(c0 + TC, T))
            finish(0)
            stage_prep(1, [(0, T)])
            stage_bc(1)
            stage_prep(2, [(0, T)])
            stage_bc(2)

    nc.compile()
    return nc


def _phi():
    """f64 [GRID, BINS] map: cell-averaged smooth-histogram contribution."""
    b = np.arange(BINS, dtype=np.float64)
    step = SCALE / 255.0
    u_grid = np.arange(GRID, dtype=np.float64)
    nsub = 17
    offs = np.linspace(-0.5, 0.5, nsub)
    wts = np.ones(nsub)
    wts[1:-1:2], wts[2:-1:2] = 4.0, 2.0
    wts /= wts.sum()
    phi = np.zeros((GRID, BINS))
    for o, ws in zip(offs, wts):
        diff = ((u_grid + o)[:, None] - step * b[None, :]) / SCALE
        w = np.exp(-0.5 * (diff / SIGMA) ** 2)
        phi += ws * (w / (w.sum(axis=1, keepdims=True) + 1e-8))
    return phi


def _seed_np():
    return np.ascontiguousarray(
        np.broadcast_to(np.arange(W, dtype=np.float32)[None, :], (P, W))
    )


def _get_state():
    if "nc" not in _CACHE:
        _CACHE["nc"] = _build_program()
        _CACHE["phi"] = _phi()
        _CACHE["seed"] = _seed_np()
    return _CACHE["nc"], _CACHE["phi"], _CACHE["seed"]


def _run_device(images, trace=False):
    """images: [3, IMG_PIX] f32 -> (results, counts [3, GRID] f64)."""
    nc, phi, _ = _get_state()
    in_maps = []
    for k in range(N_CORES):
        shard = images[:, k * SHARD : (k + 1) * SHARD].reshape(N_IMG, P, T)
        in_maps.append({"x": np.ascontiguousarray(shard)})
    res = run_bass_kernel_spmd(nc, in_maps, list(range(N_CORES)), trace=trace)
    cnt = np.zeros((N_IMG, GRID), dtype=np.float64)
    for k in range(N_CORES):
        ps = res.results[k]["cnt"].astype(np.float64)  # [3, 128, 128]
        # ps[8l+g, 16g+h] -> cnt[u = 16h+l]
        psr = ps.reshape(N_IMG, W, NG, NG, W)  # [i, l, g, g', h]
        for g in range(NG):
            cnt += psr[:, :, g, g, :].transpose(0, 2, 1).reshape(N_IMG, GRID)
    return res, cnt


def kernel(fused_image, ir_image, visible_gray):
    imgs = np.stack(
        [
            np.asarray(fused_image, dtype=np.float32).reshape(-1),
            np.asarray(ir_image, dtype=np.float32).reshape(-1),
            np.asarray(visible_gray, dtype=np.float32).reshape(-1),
        ]
    )
    _, cnt = _run_device(imgs)
    _, phi, _ = _get_state()
    hists = cnt @ phi  # [3, 256] f64
    hf, hi_, hv = hists
    loss_ir = np.mean((hf - hi_) ** 2)
    loss_vis = np.mean((hf - hv) ** 2)
    return np.array(0.5 * loss_ir + 0.5 * loss_vis, dtype=np.float32)
